# revision 29
# baseline (speedup 1.0000x reference)
"""Trainium2 Bass kernel for the CfC cell (nn_CfCCell), data-parallel on 8 cores.

Math (per row):
    ff1 = gelu(x_cat @ W_ff1.T + b_ff1)          x_cat = [x, hx]
    ff2 = gelu(ff1 @ W_ff2.T + b_ff2)
    t   = sigmoid(ff2 @ (W_ta+W_tb).T + b_ta+b_tb)      (TS == 1.0)
    ic  = gelu(x @ W_in.T + b_in + input_b)
    rc  = gelu(hx @ W_r.T + r_b)
    out = hx + t * (ic + rc - hx)

Device mapping: batch sharded 8 ways. Within a core, batch is processed in
megatiles of R rows. Activations are kept feature-major ([H, batch] in SBUF)
so the feature dim is the matmul contraction (partition) axis; x/hx are
transposed on the PE (identity matmul) after a casting DMA load, and the
result is transposed back before the store. sigmoid is computed as
0.5*tanh(z/2)+0.5 so that every ScalarE op (gelu, tanh) lives in the single
"gelu_and_others" activation-table set (no table reloads).
"""

from contextlib import ExitStack

import ml_dtypes
import numpy as np

import concourse.bacc as bacc
import concourse.bass as bass
import concourse.mybir as mybir
import concourse.tile as tile
from concourse import masks
from concourse.bass_utils import run_bass_kernel_spmd

AF = mybir.ActivationFunctionType
ALU = mybir.AluOpType
BF16 = mybir.dt.bfloat16
F32 = mybir.dt.float32
NP_BF16 = ml_dtypes.bfloat16

B, I, H = 131072, 128, 256
N_CORES = 8
B_CORE = B // N_CORES  # 16384
R = 2048               # megatile rows
NB = R // 128          # 16 row-blocks per megatile

# layer order; K = contraction chunks of 128
LAYERS = ("ff1", "ff2", "tab", "ic", "rc")
KCH = {"ff1": 3, "ff2": 2, "tab": 2, "ic": 1, "rc": 2}
W_BASE = {}
_acc = 0
for _l in LAYERS:
    W_BASE[_l] = _acc
    _acc += KCH[_l] * 2
N_WCH = _acc  # 20 weight chunks of [128, 128]
BIAS_COL = {(_l, _m): 2 * _i + _m for _i, _l in enumerate(LAYERS) for _m in range(2)}


def build_nc(b_core: int = B_CORE, r: int = R) -> bass.Bass:
    nb = r // 128
    nm = b_core // r
    assert b_core % r == 0 and r % 1024 == 0

    nc = bacc.Bacc("TRN2")
    x_d = nc.dram_tensor("x", [b_core, I], F32, kind="ExternalInput")
    hx_d = nc.dram_tensor("hx", [b_core, H], F32, kind="ExternalInput")
    w_d = nc.dram_tensor("wstack", [N_WCH, 128, 128], BF16, kind="ExternalInput")
    b_d = nc.dram_tensor("bstack", [128, 10], F32, kind="ExternalInput")
    out_d = nc.dram_tensor("out", [b_core, H], F32, kind="ExternalOutput")

    with tile.TileContext(nc) as tc, ExitStack() as ctx:
        const = ctx.enter_context(tc.tile_pool(name="const", bufs=1))
        w_sb = const.tile([128, N_WCH * 128], BF16)
        nc.sync.dma_start(
            w_sb[:].rearrange("p (c f) -> p c f", c=N_WCH),
            w_d[:].rearrange("c p f -> p c f"))
        b_sb = const.tile([128, 10], F32)
        nc.sync.dma_start(b_sb[:], b_d[:])
        ident = const.tile([128, 128], BF16)
        masks.make_identity(nc, ident[:])

        io = ctx.enter_context(tc.tile_pool(name="io", bufs=2))
        acts = ctx.enter_context(tc.tile_pool(name="acts", bufs=2))
        tmp = ctx.enter_context(tc.tile_pool(name="tmp", bufs=2))
        ps_mm = ctx.enter_context(tc.tile_pool(name="ps_mm", bufs=3, space="PSUM"))
        ps_tp = ctx.enter_context(tc.tile_pool(name="ps_tp", bufs=2, space="PSUM"))

        xd, hxd, outd = x_d[:], hx_d[:], out_d[:]

        def wchunk(layer, k, m):
            ci = W_BASE[layer] + 2 * k + m
            return w_sb[:, ci * 128:(ci + 1) * 128]

        def stage_a(r0, rt, first):
            """Loads, input transposes, and all matmul+activation layers for
            rows [r0, r0+rt)."""
            nbt = rt // 128
            # load (fp32 -> bf16 cast in DMA), batch-major; interleave x/hx
            # chunks so the first transpose packs can start before the whole
            # megatile lands (matters most for the ramp-up tile)
            x_nat = io.tile([128, nb * I], BF16, tag="x_nat")
            hx_nat = io.tile([128, nb * H], BF16, tag="hx_nat")
            ng = 4 if first else 2
            g_nb = nbt // ng
            for g in range(ng):
                rows = slice(r0 + g * (rt // ng), r0 + (g + 1) * (rt // ng))
                nc.gpsimd.dma_start(
                    x_nat[:, g * g_nb * I:(g + 1) * g_nb * I].rearrange(
                        "p (n f) -> p n f", n=g_nb),
                    xd[rows, :].rearrange("(n p) f -> p n f", p=128))
                nc.gpsimd.dma_start(
                    hx_nat[:, g * g_nb * H:(g + 1) * g_nb * H].rearrange(
                        "p (n f) -> p n f", n=g_nb),
                    hxd[rows, :].rearrange("(n p) f -> p n f", p=128))

            # transpose to feature-major via PE, q-major so the h=0 layer
            # units can start after the first three packs
            xT = acts.tile([128, r], BF16, tag="xT")
            hxT = acts.tile([128, 2 * r], BF16, tag="hxT")  # [c, batch] flat
            for q in range(nbt // 8):
                tp = ps_tp.tile([128, 1024], BF16, tag="tp")
                for i2 in range(8):
                    n = 8 * q + i2
                    nc.tensor.transpose(
                        tp[:, i2 * 128:(i2 + 1) * 128],
                        x_nat[:, n * I:(n + 1) * I], ident[:])
                nc.vector.tensor_copy(xT[:, q * 1024:(q + 1) * 1024], tp[:])
                for c in range(2):
                    tp = ps_tp.tile([128, 1024], BF16, tag="tp")
                    for i2 in range(8):
                        n = 8 * q + i2
                        nc.tensor.transpose(
                            tp[:, i2 * 128:(i2 + 1) * 128],
                            hx_nat[:, n * H + c * 128:n * H + (c + 1) * 128],
                            ident[:])
                    nc.vector.tensor_copy(
                        hxT[:, c * rt + q * 1024:c * rt + (q + 1) * 1024], tp[:])

            def layer_unit(layer, srcs, func, scale, out_tile):
                K = KCH[layer]
                for m in range(2):
                    col = BIAS_COL[(layer, m)]
                    for h in range(rt // 1024):
                        ps = ps_mm.tile([128, 1024], F32, tag="mm")
                        for j in range(2):
                            sl = slice(h * 1024 + j * 512, h * 1024 + (j + 1) * 512)
                            for k in range(K):
                                nc.tensor.matmul(
                                    ps[:, j * 512:(j + 1) * 512],
                                    wchunk(layer, k, m),
                                    srcs[k][:, sl],
                                    start=(k == 0), stop=(k == K - 1))
                        nc.scalar.activation(
                            out_tile[:, m * rt + h * 1024:m * rt + (h + 1) * 1024],
                            ps[:], func, bias=b_sb[:, col:col + 1], scale=scale)

            hxT0, hxT1 = hxT[:, 0:rt], hxT[:, rt:2 * rt]
            # ic/rc are independent of ff1/ff2 — run them while ACT drains ff1
            ff1 = acts.tile([128, 2 * r], BF16, tag="ff1")
            layer_unit("ff1", [xT, hxT0, hxT1], AF.Gelu, 1.0, ff1)
            ic = acts.tile([128, 2 * r], BF16, tag="ic")
            layer_unit("ic", [xT], AF.Gelu, 1.0, ic)
            rc = acts.tile([128, 2 * r], BF16, tag="rc")
            layer_unit("rc", [hxT0, hxT1], AF.Gelu, 1.0, rc)
            ff2 = acts.tile([128, 2 * r], BF16, tag="ff2")
            layer_unit("ff2", [ff1[:, 0:rt], ff1[:, rt:2 * rt]], AF.Gelu, 1.0, ff2)
            u = acts.tile([128, 2 * r], BF16, tag="u")
            layer_unit("tab", [ff2[:, 0:rt], ff2[:, rt:2 * rt]], AF.Tanh, 0.5, u)
            return {"r0": r0, "rt": rt, "hxT": hxT, "u": u, "ic": ic, "rc": rc}

        def stage_b(st):
            """Combine on DVE, transpose back on PE, store."""
            r0, rt = st["r0"], st["rt"]
            hxT, u, ic, rc = st["hxT"], st["u"], st["ic"], st["rc"]
            nbt = rt // 128
            # out = hx + t_interp*(ic+rc-hx);  t_interp = 0.5*u + 0.5
            hT = acts.tile([128, 2 * r], BF16, tag="hT")
            for m in range(2):
                msl = slice(m * rt, (m + 1) * rt)
                ti = tmp.tile([128, r], BF16, tag="ti")
                ti = ti[:, 0:rt]
                nc.vector.tensor_scalar(
                    ti, u[:, msl], 0.5, 0.5, ALU.mult, ALU.add)
                s = tmp.tile([128, r], BF16, tag="s")
                s = s[:, 0:rt]
                nc.vector.tensor_add(s, ic[:, msl], rc[:, msl])
                d = tmp.tile([128, r], BF16, tag="d")
                d = d[:, 0:rt]
                nc.vector.tensor_sub(d, s, hxT[:, msl])
                p = tmp.tile([128, r], BF16, tag="p")
                p = p[:, 0:rt]
                nc.vector.tensor_mul(p, ti, d)
                nc.vector.tensor_add(hT[:, msl], p, hxT[:, msl])

            # transpose back per quarter and kick each store immediately so
            # the final DMA isn't one big end-of-kernel lump
            out_nat = io.tile([128, nb * H], BF16, tag="out_nat")
            for q in range(nbt // 4):
                tp = ps_tp.tile([128, 1024], BF16, tag="tp")
                for i2 in range(4):
                    n = 4 * q + i2
                    for m in range(2):
                        nc.tensor.transpose(
                            tp[:, i2 * 256 + m * 128:i2 * 256 + (m + 1) * 128],
                            hT[:, m * rt + n * 128:m * rt + (n + 1) * 128],
                            ident[:])
                nc.vector.tensor_copy(
                    out_nat[:, q * 1024:(q + 1) * 1024], tp[:])
                rows = slice(r0 + q * 512, r0 + (q + 1) * 512)
                nc.gpsimd.dma_start(
                    outd[rows, :].rearrange("(n p) f -> p n f", p=128),
                    out_nat[:, q * 1024:(q + 1) * 1024].rearrange(
                        "p (n f) -> p n f", n=4))

        # megatile schedule: small first tile (fast pipeline fill) and small
        # last tile (short drain tail), full-size tiles in the middle
        if b_core > 2 * r:
            sizes = [1024] + [r] * ((b_core - 2048) // r) + [1024]
        else:
            sizes = [r] * (b_core // r)
        assert sum(sizes) == b_core

        # software pipeline: defer each megatile's output stage until after
        # the next megatile's matmul work is queued, so the PE never sits
        # behind the DVE combine tail (keeps HAM warm across boundaries).
        prev = None
        r0 = 0
        for ti_, rt in enumerate(sizes):
            st = stage_a(r0, rt, ti_ == 0)
            r0 += rt
            if prev is not None:
                stage_b(prev)
            prev = st
        stage_b(prev)
    nc.finalize()
    return nc


_NC_CACHE: dict = {}


def _get_nc(b_core: int, r: int) -> bass.Bass:
    key = (b_core, r)
    if key not in _NC_CACHE:
        _NC_CACHE[key] = build_nc(b_core, r)
    return _NC_CACHE[key]


def _prep_host(W_ff1, b_ff1, W_ff2, b_ff2, W_ta, b_ta, W_tb, b_tb,
               W_in, b_in, input_b, W_r, r_b):
    f32 = lambda a: np.asarray(a, dtype=np.float32)
    weights = {
        "ff1": f32(W_ff1),
        "ff2": f32(W_ff2),
        "tab": f32(W_ta) + f32(W_tb),
        "ic": f32(W_in),
        "rc": f32(W_r),
    }
    biases = {
        "ff1": f32(b_ff1),
        "ff2": f32(b_ff2),
        "tab": 0.5 * (f32(b_ta) + f32(b_tb)),
        "ic": f32(b_in) + f32(input_b),
        "rc": f32(r_b),
    }
    wstack = np.zeros([N_WCH, 128, 128], dtype=NP_BF16)
    for layer in LAYERS:
        W = weights[layer]
        for k in range(KCH[layer]):
            for m in range(2):
                ci = W_BASE[layer] + 2 * k + m
                wstack[ci] = np.ascontiguousarray(
                    W[m * 128:(m + 1) * 128, k * 128:(k + 1) * 128].T
                ).astype(NP_BF16)
    bstack = np.zeros([128, 10], dtype=np.float32)
    for li, layer in enumerate(LAYERS):
        for m in range(2):
            bstack[:, 2 * li + m] = biases[layer][m * 128:(m + 1) * 128]
    return wstack, bstack


def _run(inputs: dict, b_core: int = B_CORE, r: int = R, n_cores: int = N_CORES,
         **run_kwargs):
    x = np.asarray(inputs["x"], dtype=np.float32)
    hx = np.asarray(inputs["hx"], dtype=np.float32)
    wstack, bstack = _prep_host(
        inputs["W_ff1"], inputs["b_ff1"], inputs["W_ff2"], inputs["b_ff2"],
        inputs["W_ta"], inputs["b_ta"], inputs["W_tb"], inputs["b_tb"],
        inputs["W_in"], inputs["b_in"], inputs["input_b"], inputs["W_r"],
        inputs["r_b"])
    nc = _get_nc(b_core, r)
    in_maps = []
    for c in range(n_cores):
        sl = slice(c * b_core, (c + 1) * b_core)
        in_maps.append({
            "x": np.ascontiguousarray(x[sl]),
            "hx": np.ascontiguousarray(hx[sl]),
            "wstack": wstack,
            "bstack": bstack,
        })
    res = run_bass_kernel_spmd(nc, in_maps, list(range(n_cores)), **run_kwargs)
    out = np.concatenate([m["out"] for m in res.results], axis=0)
    return out, res


def kernel(**inputs):
    out, _ = _run(inputs)
    return (out, out)


# revision 33
# speedup vs baseline: 1.0052x; 1.0052x over previous
"""Trainium2 Bass kernel for the CfC cell (nn_CfCCell), data-parallel on 8 cores.

Math (per row):
    ff1 = gelu(x_cat @ W_ff1.T + b_ff1)          x_cat = [x, hx]
    ff2 = gelu(ff1 @ W_ff2.T + b_ff2)
    t   = sigmoid(ff2 @ (W_ta+W_tb).T + b_ta+b_tb)      (TS == 1.0)
    ic  = gelu(x @ W_in.T + b_in + input_b)
    rc  = gelu(hx @ W_r.T + r_b)
    out = hx + t * (ic + rc - hx)

Device mapping: batch sharded 8 ways. Within a core, batch is processed in
megatiles of R rows. Activations are kept feature-major ([H, batch] in SBUF)
so the feature dim is the matmul contraction (partition) axis; x/hx are
transposed on the PE (identity matmul) after a casting DMA load, and the
result is transposed back before the store. sigmoid is computed as
0.5*tanh(z/2)+0.5 so that every ScalarE op (gelu, tanh) lives in the single
"gelu_and_others" activation-table set (no table reloads).
"""

from contextlib import ExitStack

import ml_dtypes
import numpy as np

import concourse.bacc as bacc
import concourse.bass as bass
import concourse.mybir as mybir
import concourse.tile as tile
from concourse import masks
from concourse.bass_utils import run_bass_kernel_spmd

AF = mybir.ActivationFunctionType
ALU = mybir.AluOpType
BF16 = mybir.dt.bfloat16
F32 = mybir.dt.float32
NP_BF16 = ml_dtypes.bfloat16

B, I, H = 131072, 128, 256
N_CORES = 8
B_CORE = B // N_CORES  # 16384
R = 2048               # megatile rows
NB = R // 128          # 16 row-blocks per megatile

# layer order; K = contraction chunks of 128
LAYERS = ("ff1", "ff2", "tab", "ic", "rc")
KCH = {"ff1": 3, "ff2": 2, "tab": 2, "ic": 1, "rc": 2}
W_BASE = {}
_acc = 0
for _l in LAYERS:
    W_BASE[_l] = _acc
    _acc += KCH[_l] * 2
N_WCH = _acc  # 20 weight chunks of [128, 128]
BIAS_COL = {(_l, _m): 2 * _i + _m for _i, _l in enumerate(LAYERS) for _m in range(2)}


def build_nc(b_core: int = B_CORE, r: int = R) -> bass.Bass:
    nb = r // 128
    nm = b_core // r
    assert b_core % r == 0 and r % 1024 == 0

    nc = bacc.Bacc("TRN2")
    x_d = nc.dram_tensor("x", [b_core, I], F32, kind="ExternalInput")
    hx_d = nc.dram_tensor("hx", [b_core, H], F32, kind="ExternalInput")
    w_d = nc.dram_tensor("wstack", [N_WCH, 128, 128], BF16, kind="ExternalInput")
    b_d = nc.dram_tensor("bstack", [128, 10], F32, kind="ExternalInput")
    out_d = nc.dram_tensor("out", [b_core, H], F32, kind="ExternalOutput")

    with tile.TileContext(nc) as tc, ExitStack() as ctx:
        const = ctx.enter_context(tc.tile_pool(name="const", bufs=1))
        w_sb = const.tile([128, N_WCH * 128], BF16)
        nc.sync.dma_start(
            w_sb[:].rearrange("p (c f) -> p c f", c=N_WCH),
            w_d[:].rearrange("c p f -> p c f"))
        b_sb = const.tile([128, 10], F32)
        nc.sync.dma_start(b_sb[:], b_d[:])
        ident = const.tile([128, 128], BF16)
        masks.make_identity(nc, ident[:])

        io = ctx.enter_context(tc.tile_pool(name="io", bufs=2))
        acts = ctx.enter_context(tc.tile_pool(name="acts", bufs=2))
        tmp = ctx.enter_context(tc.tile_pool(name="tmp", bufs=2))
        ps_mm = ctx.enter_context(tc.tile_pool(name="ps_mm", bufs=3, space="PSUM"))
        ps_tp = ctx.enter_context(tc.tile_pool(name="ps_tp", bufs=2, space="PSUM"))

        # HAM warm-up: ~3.5us of dummy PE work while the first loads land, so
        # the first real transposes/matmuls run at 2.4 GHz instead of 1.2
        warm = ps_tp.tile([128, 1024], BF16, tag="tp")
        for i in range(32):
            nc.tensor.transpose(
                warm[:, (i % 8) * 128:(i % 8 + 1) * 128], ident[:], ident[:])

        xd, hxd, outd = x_d[:], hx_d[:], out_d[:]

        def wchunk(layer, k, m):
            ci = W_BASE[layer] + 2 * k + m
            return w_sb[:, ci * 128:(ci + 1) * 128]

        def stage_a(r0, rt, first):
            """Loads, input transposes, and all matmul+activation layers for
            rows [r0, r0+rt)."""
            nbt = rt // 128
            # load (fp32 -> bf16 cast in DMA), batch-major; interleave x/hx
            # chunks so the first transpose packs can start before the whole
            # megatile lands (matters most for the ramp-up tile)
            x_nat = io.tile([128, nb * I], BF16, tag="x_nat")
            hx_nat = io.tile([128, nb * H], BF16, tag="hx_nat")
            ng = 4 if first else 2
            g_nb = nbt // ng
            for g in range(ng):
                rows = slice(r0 + g * (rt // ng), r0 + (g + 1) * (rt // ng))
                nc.gpsimd.dma_start(
                    x_nat[:, g * g_nb * I:(g + 1) * g_nb * I].rearrange(
                        "p (n f) -> p n f", n=g_nb),
                    xd[rows, :].rearrange("(n p) f -> p n f", p=128))
                nc.gpsimd.dma_start(
                    hx_nat[:, g * g_nb * H:(g + 1) * g_nb * H].rearrange(
                        "p (n f) -> p n f", n=g_nb),
                    hxd[rows, :].rearrange("(n p) f -> p n f", p=128))

            # transpose to feature-major via PE, q-major so the h=0 layer
            # units can start after the first three packs; small packs on the
            # first tile so transposes overlap the initial load
            pb = 2 if first else 8
            xT = acts.tile([128, r], BF16, tag="xT")
            hxT = acts.tile([128, 2 * r], BF16, tag="hxT")  # [c, batch] flat
            for q in range(nbt // pb):
                cs = pb * q * 128
                tp = ps_tp.tile([128, 1024], BF16, tag="tp")
                for i2 in range(pb):
                    n = pb * q + i2
                    nc.tensor.transpose(
                        tp[:, i2 * 128:(i2 + 1) * 128],
                        x_nat[:, n * I:(n + 1) * I], ident[:])
                nc.vector.tensor_copy(
                    xT[:, cs:cs + pb * 128], tp[:, 0:pb * 128])
                for c in range(2):
                    tp = ps_tp.tile([128, 1024], BF16, tag="tp")
                    for i2 in range(pb):
                        n = pb * q + i2
                        nc.tensor.transpose(
                            tp[:, i2 * 128:(i2 + 1) * 128],
                            hx_nat[:, n * H + c * 128:n * H + (c + 1) * 128],
                            ident[:])
                    nc.vector.tensor_copy(
                        hxT[:, c * rt + cs:c * rt + cs + pb * 128],
                        tp[:, 0:pb * 128])

            def layer_unit(layer, srcs, func, scale, out_tile):
                K = KCH[layer]
                for m in range(2):
                    col = BIAS_COL[(layer, m)]
                    for h in range(rt // 1024):
                        ps = ps_mm.tile([128, 1024], F32, tag="mm")
                        for j in range(2):
                            sl = slice(h * 1024 + j * 512, h * 1024 + (j + 1) * 512)
                            for k in range(K):
                                nc.tensor.matmul(
                                    ps[:, j * 512:(j + 1) * 512],
                                    wchunk(layer, k, m),
                                    srcs[k][:, sl],
                                    start=(k == 0), stop=(k == K - 1))
                        nc.scalar.activation(
                            out_tile[:, m * rt + h * 1024:m * rt + (h + 1) * 1024],
                            ps[:], func, bias=b_sb[:, col:col + 1], scale=scale)

            hxT0, hxT1 = hxT[:, 0:rt], hxT[:, rt:2 * rt]
            # ic/rc are independent of ff1/ff2 — run them while ACT drains ff1
            ff1 = acts.tile([128, 2 * r], BF16, tag="ff1")
            layer_unit("ff1", [xT, hxT0, hxT1], AF.Gelu, 1.0, ff1)
            ic = acts.tile([128, 2 * r], BF16, tag="ic")
            layer_unit("ic", [xT], AF.Gelu, 1.0, ic)
            rc = acts.tile([128, 2 * r], BF16, tag="rc")
            layer_unit("rc", [hxT0, hxT1], AF.Gelu, 1.0, rc)
            ff2 = acts.tile([128, 2 * r], BF16, tag="ff2")
            layer_unit("ff2", [ff1[:, 0:rt], ff1[:, rt:2 * rt]], AF.Gelu, 1.0, ff2)
            u = acts.tile([128, 2 * r], BF16, tag="u")
            layer_unit("tab", [ff2[:, 0:rt], ff2[:, rt:2 * rt]], AF.Tanh, 0.5, u)
            return {"r0": r0, "rt": rt, "hxT": hxT, "u": u, "ic": ic, "rc": rc}

        def stage_b(st):
            """Combine on DVE, transpose back on PE, store."""
            r0, rt = st["r0"], st["rt"]
            hxT, u, ic, rc = st["hxT"], st["u"], st["ic"], st["rc"]
            nbt = rt // 128
            # out = hx + t_interp*(ic+rc-hx);  t_interp = 0.5*u + 0.5
            hT = acts.tile([128, 2 * r], BF16, tag="hT")
            for m in range(2):
                msl = slice(m * rt, (m + 1) * rt)
                ti = tmp.tile([128, r], BF16, tag="ti")
                ti = ti[:, 0:rt]
                nc.vector.tensor_scalar(
                    ti, u[:, msl], 0.5, 0.5, ALU.mult, ALU.add)
                s = tmp.tile([128, r], BF16, tag="s")
                s = s[:, 0:rt]
                nc.vector.tensor_add(s, ic[:, msl], rc[:, msl])
                d = tmp.tile([128, r], BF16, tag="d")
                d = d[:, 0:rt]
                nc.vector.tensor_sub(d, s, hxT[:, msl])
                p = tmp.tile([128, r], BF16, tag="p")
                p = p[:, 0:rt]
                nc.vector.tensor_mul(p, ti, d)
                nc.vector.tensor_add(hT[:, msl], p, hxT[:, msl])

            # transpose back per quarter and kick each store immediately so
            # the final DMA isn't one big end-of-kernel lump
            out_nat = io.tile([128, nb * H], BF16, tag="out_nat")
            for q in range(nbt // 4):
                tp = ps_tp.tile([128, 1024], BF16, tag="tp")
                for i2 in range(4):
                    n = 4 * q + i2
                    for m in range(2):
                        nc.tensor.transpose(
                            tp[:, i2 * 256 + m * 128:i2 * 256 + (m + 1) * 128],
                            hT[:, m * rt + n * 128:m * rt + (n + 1) * 128],
                            ident[:])
                nc.vector.tensor_copy(
                    out_nat[:, q * 1024:(q + 1) * 1024], tp[:])
                rows = slice(r0 + q * 512, r0 + (q + 1) * 512)
                nc.gpsimd.dma_start(
                    outd[rows, :].rearrange("(n p) f -> p n f", p=128),
                    out_nat[:, q * 1024:(q + 1) * 1024].rearrange(
                        "p (n f) -> p n f", n=4))

        # megatile schedule: small first tile (fast pipeline fill) and small
        # last tile (short drain tail), full-size tiles in the middle
        if b_core > 2 * r:
            sizes = [1024] + [r] * ((b_core - 2048) // r) + [1024]
        else:
            sizes = [r] * (b_core // r)
        assert sum(sizes) == b_core

        # software pipeline: defer each megatile's output stage until after
        # the next megatile's matmul work is queued, so the PE never sits
        # behind the DVE combine tail (keeps HAM warm across boundaries).
        prev = None
        r0 = 0
        for ti_, rt in enumerate(sizes):
            st = stage_a(r0, rt, ti_ == 0)
            r0 += rt
            if prev is not None:
                stage_b(prev)
            prev = st
        stage_b(prev)
    nc.finalize()
    return nc


_NC_CACHE: dict = {}


def _get_nc(b_core: int, r: int) -> bass.Bass:
    key = (b_core, r)
    if key not in _NC_CACHE:
        _NC_CACHE[key] = build_nc(b_core, r)
    return _NC_CACHE[key]


def _prep_host(W_ff1, b_ff1, W_ff2, b_ff2, W_ta, b_ta, W_tb, b_tb,
               W_in, b_in, input_b, W_r, r_b):
    f32 = lambda a: np.asarray(a, dtype=np.float32)
    weights = {
        "ff1": f32(W_ff1),
        "ff2": f32(W_ff2),
        "tab": f32(W_ta) + f32(W_tb),
        "ic": f32(W_in),
        "rc": f32(W_r),
    }
    biases = {
        "ff1": f32(b_ff1),
        "ff2": f32(b_ff2),
        "tab": 0.5 * (f32(b_ta) + f32(b_tb)),
        "ic": f32(b_in) + f32(input_b),
        "rc": f32(r_b),
    }
    wstack = np.zeros([N_WCH, 128, 128], dtype=NP_BF16)
    for layer in LAYERS:
        W = weights[layer]
        for k in range(KCH[layer]):
            for m in range(2):
                ci = W_BASE[layer] + 2 * k + m
                wstack[ci] = np.ascontiguousarray(
                    W[m * 128:(m + 1) * 128, k * 128:(k + 1) * 128].T
                ).astype(NP_BF16)
    bstack = np.zeros([128, 10], dtype=np.float32)
    for li, layer in enumerate(LAYERS):
        for m in range(2):
            bstack[:, 2 * li + m] = biases[layer][m * 128:(m + 1) * 128]
    return wstack, bstack


def _run(inputs: dict, b_core: int = B_CORE, r: int = R, n_cores: int = N_CORES,
         **run_kwargs):
    x = np.asarray(inputs["x"], dtype=np.float32)
    hx = np.asarray(inputs["hx"], dtype=np.float32)
    wstack, bstack = _prep_host(
        inputs["W_ff1"], inputs["b_ff1"], inputs["W_ff2"], inputs["b_ff2"],
        inputs["W_ta"], inputs["b_ta"], inputs["W_tb"], inputs["b_tb"],
        inputs["W_in"], inputs["b_in"], inputs["input_b"], inputs["W_r"],
        inputs["r_b"])
    nc = _get_nc(b_core, r)
    in_maps = []
    for c in range(n_cores):
        sl = slice(c * b_core, (c + 1) * b_core)
        in_maps.append({
            "x": np.ascontiguousarray(x[sl]),
            "hx": np.ascontiguousarray(hx[sl]),
            "wstack": wstack,
            "bstack": bstack,
        })
    res = run_bass_kernel_spmd(nc, in_maps, list(range(n_cores)), **run_kwargs)
    out = np.concatenate([m["out"] for m in res.results], axis=0)
    return out, res


def kernel(**inputs):
    out, _ = _run(inputs)
    return (out, out)
